# revision 45
# baseline (speedup 1.0000x reference)
"""Multi-head attention (b=4, n=2048, d=768, h=12) on 8 trn2 NeuronCores.

Sharding: (batch x head-half) -> 8 shards. Each core gets one batch's x
and SIX of the twelve heads (3 head pairs), computes Q/K/V for the full
sequence for just those heads, runs attention + its slice of the output
projection for the full 2048 queries, and returns a partial [2048, 768]
output (bias folded into the even core of each pair). Host sums the two
partials per batch. No collectives, no duplicated K/V compute.

Device algorithm (per core); storage bf16 for all matmul operands (PSUM
accumulation and softmax stay fp32; end-to-end rel err ~2e-3):
  1. x^T supplied pre-transposed (and pre-cast to bf16) by the host.
  2. Per local head pair: K^T/Q^T/V^T = (x @ W)^T via lhsT=W chunks
     ([128, 2048] each); V^T is PE-transposed back to natural [n, d]
     layout (+ ones column per head for the softmax denominators).
  3. Per head, per query-half qh: S^T[k,q] = K^T_slice.T @ Q^T slices,
     P^T = exp(S^T * scale) on ACT (bf16 out), O^T += V_aug.T @ P^T into
     two 1-bank PSUM halves [65,512]. Row 64 of O^T = denominators.
  4. Projection per (head, qh, qblock): denominators land on partitions
     via a K=1 matmul of the O^T denom row, then
     acc[qb] += r_q * (O_un,h @ W_h) via fused scalar_tensor_tensor.
     The first head accumulates onto the broadcast bias, so acc needs no
     seeding and the epilogue is just 16 output DMAs that stream out as
     the last head's projection completes.

Scheduling: the next pair's QKV and the previous (head, qh)'s
projection are chopped into <=1.3us "filler" chunks drained one per
kt-iteration, and O(kt-1) is emitted after S(kt)/exp(kt) (software
pipeline), so the PE works through the exp latency instead of stalling
on it; ACT (exp, the per-head limiter) never starves. TimelineSim:
298us/core vs 353us for the fp32r batch x seq-half baseline.
"""

from collections import deque

import numpy as np

B, N, D, H, HD = 4, 2048, 768, 12, 64
SCALE = HD ** -0.5
NCORES = 8
NT = N // 128           # 16 key tiles
DT = D // 128           # 6 d-chunks
QT = N // 128           # 16 query blocks (full seq per core)
NPL = 3                 # local head pairs per core (6 heads)
WCOLS = NPL * 128       # 384 packed weight columns per section

_RUNNER = None


def _build_program(reps=1):
    import concourse.bass as bass
    import concourse.tile as tile
    import concourse.mybir as mybir
    from concourse import bacc
    from concourse.masks import make_identity
    from contextlib import ExitStack

    f32 = mybir.dt.float32
    bf16 = mybir.dt.bfloat16
    AF = mybir.ActivationFunctionType
    ALU = mybir.AluOpType

    nc = bacc.Bacc("TRN2", target_bir_lowering=False, debug=False,
                   num_devices=NCORES)

    # host passes x^T bf16; per-core packed qkv weights [D, 3*384]
    # (q cols | k cols | v cols for this core's 3 pairs); w_proj rows for
    # this core's heads [384, D]; bias (real on even cores, zeros on odd).
    xt_in = nc.dram_tensor("xt", [D, N], bf16, kind="ExternalInput")
    w_qkv = nc.dram_tensor("wqkv", [D, 3 * WCOLS], bf16, kind="ExternalInput")
    w_proj = nc.dram_tensor("wproj", [WCOLS, D], bf16, kind="ExternalInput")
    b_proj = nc.dram_tensor("bias", [D], f32, kind="ExternalInput")
    y = nc.dram_tensor("y", [N, D], f32, kind="ExternalOutput")

    with tile.TileContext(nc) as tc:
      for _rep in range(reps):
        with ExitStack() as ctx:
          singles = ctx.enter_context(tc.tile_pool(name="singles", bufs=1))
          onat_pool = ctx.enter_context(tc.tile_pool(name="onat", bufs=1))
          small_pool = ctx.enter_context(tc.tile_pool(name="small", bufs=4))
          # psum pool for transposes / QKV accumulation / proj (2 banks)
          ps_misc = ctx.enter_context(
              tc.tile_pool(name="ps_misc", bufs=2, space="PSUM"))

          ident = singles.tile([128, 128], bf16)
          make_identity(nc, ident)

          bias_bc = singles.tile([128, D], f32)

          ones128 = singles.tile([128, 2], bf16)
          nc.vector.memset(ones128[:], 1.0)

          # warm the ACT exp table during the startup DMA window
          warm = small_pool.tile([128, 1], f32, tag="warm", name="warm")
          nc.vector.memset(warm[:], 0.0)
          nc.scalar.activation(warm[:], warm[:], AF.Exp, bias=0.0, scale=1.0)

          # w_proj pair-row tiles [128, D]; DMAs emitted after the x^T
          # loads (see startup section). The first head's projection
          # accumulates onto bias_bc directly, so acc needs no seeding.
          wp = [singles.tile([128, D], bf16, tag=f"wp{p}", name=f"wp{p}")
                for p in range(NPL)]
          acc = [onat_pool.tile([128, D], f32, tag=f"acc{qb}", name=f"acc{qb}")
                 for qb in range(QT)]

          with tc.tile_pool(name="xt", bufs=1) as xt_pool:
              xT = [xt_pool.tile([128, N], bf16, tag=f"xt{dt}", name=f"xt{dt}")
                    for dt in range(DT)]

              with tc.tile_pool(name="wkq", bufs=16) as wkq_pool, \
                   tc.tile_pool(name="kq", bufs=2) as kq_pool, \
                   tc.tile_pool(name="vsb", bufs=2) as vsb_pool, \
                   tc.tile_pool(name="pt", bufs=3) as pt_pool, \
                   tc.tile_pool(name="otsb", bufs=3) as ot_pool:
                  ps_att = ExitStack()
                  ps_st = ps_att.enter_context(
                      tc.tile_pool(name="ps_st", bufs=2, space="PSUM"))
                  ps_ot = ps_att.enter_context(
                      tc.tile_pool(name="ps_ot", bufs=1, space="PSUM"))
                  # proj chunks allocate their PSUM from here; swapped to
                  # a deeper tail pool once the attention PSUM is freed
                  ps_proj = [ps_misc]

                  # ---- weight DMAs for one pair: one DMA per matrix,
                  # landing as [128, DT, 128] (partition = row-within-
                  # d-chunk) so lhsT slices are t[:, dt, :] ----
                  wq_ap = w_qkv.ap()

                  def w_src(col0, dt0, ndt):
                      return bass.AP(
                          tensor=wq_ap.tensor,
                          offset=wq_ap.offset + col0 + dt0 * 128 * 3 * WCOLS,
                          ap=[[3 * WCOLS, 128],
                              [128 * 3 * WCOLS, ndt],
                              [1, 128]])

                  def emit_w(p):
                      out = []
                      for (tag, col0), eng in zip(
                              (("wk", WCOLS + p * 128),
                               ("wq", p * 128),
                               ("wv", 2 * WCOLS + p * 128)),
                              (nc.sync, nc.gpsimd, nc.sync)):
                          t = wkq_pool.tile([128, DT, 128], bf16, tag=tag,
                                            name=tag)
                          eng.dma_start(out=t[:], in_=w_src(col0, 0, DT))
                          out.append(lambda dt, t=t: t[:, dt, :])
                      wk, wq, wv = out
                      return wk, wq, wv

                  # ---- one 512-col slice of a [128, 2048] (x @ W)^T ----
                  def emit_xw_slice(wf, dst, ns):
                      pss = ps_misc.tile([128, 512], f32, tag="misc",
                                         name="pss")
                      for dt in range(DT):
                          nc.tensor.matmul(
                              pss[:], lhsT=wf(dt),
                              rhs=xT[dt][:, ns * 512:(ns + 1) * 512],
                              start=(dt == 0), stop=(dt == DT - 1))
                      nc.vector.tensor_copy(
                          out=dst[:, ns * 512:(ns + 1) * 512], in_=pss[:])

                  # ---- V^T tile-pair transpose into natural layout ----
                  def emit_vt_chunk(VTt, Vsb, kts):
                      for kt in kts:
                          ps = ps_misc.tile([128, 128], bf16, tag="misc")
                          nc.tensor.transpose(
                              ps[:], VTt[:, kt * 128:(kt + 1) * 128],
                              ident[:])
                          dst = Vsb[:, kt, :].rearrange(
                              "n (h c) -> n h c", h=2)
                          nc.vector.tensor_copy(
                              out=dst[:, :, 0:64],
                              in_=ps[:].rearrange("n (h c) -> n h c", h=2))

                  # ---- build the QKV chunk thunks for pair p ----
                  # returns (KT, QTt, Vsb, direct-chunks, filler-chunks);
                  # direct chunks are the minimum needed before the
                  # pair's first attention kt-iterations can run, the
                  # rest interleave as fillers
                  def make_qkv(p, w3):
                      wk, wq, wv = w3
                      KT = kq_pool.tile([128, N], bf16, tag="kt")
                      QTt = kq_pool.tile([128, N], bf16, tag="qt")
                      VTt = kq_pool.tile([128, N], bf16, tag="vt")
                      Vsb = vsb_pool.tile([128, NT, 130], bf16, tag="v")

                      def K(ns):
                          return lambda: emit_xw_slice(wk, KT, ns)

                      def Q(ns):
                          return lambda: emit_xw_slice(wq, QTt, ns)

                      def V(ns):
                          return lambda: emit_xw_slice(wv, VTt, ns)

                      def T(k0):
                          return lambda: emit_vt_chunk(
                              VTt, Vsb, range(k0, k0 + 4))

                      def vsb_ones():
                          nc.vector.memset(Vsb[:, :, 64:65], 1.0)
                          nc.vector.memset(Vsb[:, :, 129:130], 1.0)

                      direct = [K(0), Q(0), Q(1), vsb_ones, V(0)]
                      fill = [T(0), K(1), V(1), T(4), K(2), V(2), T(8),
                              K(3), V(3), T(12), Q(2), Q(3)]
                      return KT, QTt, Vsb, direct, fill

                  fillers = deque()

                  def drain_filler():
                      if fillers:
                          fillers.popleft()()

                  # ---- projection of one (head, qh): 8 qblocks, chopped
                  # into 2-qblock filler chunks. The first head reads
                  # bias_bc as the accumulate input (acc is uninitialized
                  # until then); later heads accumulate onto acc. ----
                  def push_proj(p, hh, otsb, qh, final):
                      first = p == 0 and hh == 0
                      if hh == 0:
                          orows = otsb[0:64, :]
                          drow = otsb[64:65, :]
                          done = ones128[64:65, :]
                      else:
                          orows = otsb[64:128, :]
                          drow = otsb[0:1, :]
                          done = ones128[0:1, :]
                      wpt = wp[p]

                      def emit_proj_qb(qb):
                          qsl = slice(qb * 128, (qb + 1) * 128)
                          psp = ps_proj[0]
                          sden = psp.tile([128, 2], f32, tag="misc",
                                          name="sden")
                          nc.tensor.matmul(sden[:], lhsT=drow[:, qsl],
                                           rhs=done, start=True, stop=True)
                          rcp = small_pool.tile([128, 1], f32, tag="rcp")
                          nc.vector.reciprocal(rcp[:], sden[:, 0:1])
                          # PSUM is only reachable from DVE (not GPSIMD),
                          # so the normalize-accumulate stays on DVE
                          veng = nc.vector
                          for i in range(2):
                              pp = psp.tile([128, 384], f32,
                                            tag="misc", name="pp")
                              nc.tensor.matmul(
                                  pp[:], lhsT=orows[:, qsl],
                                  rhs=wpt[hh * 64:(hh + 1) * 64,
                                          i * 384:(i + 1) * 384],
                                  start=True, stop=True)
                              src = (bias_bc if first else acc[qb])
                              veng.scalar_tensor_tensor(
                                  out=acc[qb][:, i * 384:(i + 1) * 384],
                                  in0=pp[:], scalar=rcp[:],
                                  in1=src[:, i * 384:(i + 1) * 384],
                                  op0=ALU.mult, op1=ALU.add)
                          if final:
                              eng = (nc.sync, nc.scalar, nc.gpsimd)[qb % 3]
                              eng.dma_start(out=y[qb * 128:(qb + 1) * 128, :],
                                            in_=acc[qb][:])

                      for qb0 in range(qh * 8, qh * 8 + 8):
                          fillers.append(
                              lambda qb0=qb0: emit_proj_qb(qb0))

                  # ---- attention for one head; interleaves fillers ----
                  def emit_attn_head(p, hh, KT, QTt, Vsb, final):
                      base = hh * 64
                      otsb = ot_pool.tile([128, N], bf16, tag="otsb")
                      for qh in range(2):
                          # O^T accumulates into two 1-bank halves so the
                          # next qh's accumulation only waits on the
                          # matching half's copy-out
                          ot0 = ps_ot.tile([65, 512], f32, tag="ot0",
                                           name="ot0")
                          ot1 = ps_ot.tile([65, 512], f32, tag="ot1",
                                           name="ot1")
                          ot = [ot0, ot1]
                          # software-pipelined kt loop: O(kt-1) is emitted
                          # after S(kt)/exp(kt) and the filler, so the PE
                          # works through the exp latency instead of
                          # stalling on it every iteration
                          prev_o = None
                          for kt in range(NT):
                              st = ps_st.tile([128, N // 2], f32, tag="st")
                              lhsT = KT[base:base + 64,
                                        kt * 128:(kt + 1) * 128]
                              for i in range(2):
                                  q0 = qh * 1024 + i * 512
                                  nc.tensor.matmul(
                                      st[:, i * 512:(i + 1) * 512],
                                      lhsT=lhsT,
                                      rhs=QTt[base:base + 64, q0:q0 + 512],
                                      start=True, stop=True)
                              pt = pt_pool.tile([128, N // 2], bf16,
                                                tag="pt")
                              nc.scalar.activation(pt[:], st[:], AF.Exp,
                                                   bias=0.0,
                                                   scale=float(SCALE))
                              if final and qh == 1:
                                  if kt >= 8:
                                      drain_filler()
                              else:
                                  drain_filler()
                              if prev_o is not None:
                                  prev_o()
                              vh = Vsb[:, kt, hh * 65: hh * 65 + 65]

                              def prev_o(pt=pt, vh=vh, kt=kt):
                                  for i in range(2):
                                      nc.tensor.matmul(
                                          ot[i][:],
                                          lhsT=vh,
                                          rhs=pt[:, i * 512:(i + 1) * 512],
                                          start=(kt == 0),
                                          stop=(kt == NT - 1))
                          prev_o()
                          # pack this qh's O^T into otsb so lhsT/rhs base
                          # partitions line up for the projection matmuls
                          # (final head, last qh: on ACT, which is idle in
                          # the tail while DVE still has proj work)
                          def ocopy(out, in_, i=0):
                              # final head's last-qh copies gate the tail
                              # projections: split them across ACT and DVE
                              # so the latency halves
                              if final and qh == 1 and i == 0:
                                  nc.scalar.activation(out, in_, AF.Copy,
                                                       bias=0.0, scale=1.0)
                              else:
                                  nc.vector.tensor_copy(out=out, in_=in_)
                          for i in range(2):
                              osl = slice(qh * 1024 + i * 512,
                                          qh * 1024 + (i + 1) * 512)
                              if hh == 0:
                                  ocopy(otsb[0:65, osl], ot[i][:], i)
                              else:
                                  ocopy(otsb[64:128, osl], ot[i][0:64, :], i)
                                  ocopy(otsb[0:1, osl], ot[i][64:65, :], i)
                          push_proj(p, hh, otsb, qh, final)

                  # ---- startup DMAs. sync + scalar share the hardware
                  # DGE (~0.63us per DMA, serial); gpsimd runs software
                  # DGE on the Pool engine (~1us per DMA) but is a third
                  # parallel path. Order: pair-0 wk, then x^T column
                  # halves (first-needed first), then wq/wv, bias, wp.
                  def w_dma(tag, col0, eng):
                      t = wkq_pool.tile([128, DT, 128], bf16, tag=tag,
                                        name=tag)
                      eng.dma_start(out=t[:], in_=w_src(col0, 0, DT))
                      return lambda dt, t=t: t[:, dt, :]

                  # pair-0 wk split: a tiny dt0-only tile lands first so
                  # the very first matmul starts earlier
                  wk0a = wkq_pool.tile([128, 1, 128], bf16, tag="wk0a",
                                       name="wk0a")
                  nc.sync.dma_start(out=wk0a[:], in_=w_src(WCOLS, 0, 1))
                  wk0b = wkq_pool.tile([128, DT - 1, 128], bf16, tag="wk",
                                       name="wk0b")
                  nc.sync.dma_start(out=wk0b[:], in_=w_src(WCOLS, 1, DT - 1))

                  def wk0t(dt):
                      return wk0a[:, 0, :] if dt == 0 else wk0b[:, dt - 1, :]

                  for half in range(2):
                      for dt in range(DT):
                          eng = (nc.gpsimd, nc.sync, nc.scalar)[dt % 3]
                          eng.dma_start(
                              out=xT[dt][:, half * 1024:(half + 1) * 1024],
                              in_=xt_in[dt * 128:(dt + 1) * 128,
                                        half * 1024:(half + 1) * 1024])
                      if half == 0:
                          wq0t = w_dma("wq", 0, nc.gpsimd)
                          wv0t = w_dma("wv", 2 * WCOLS, nc.gpsimd)
                  b_ap = b_proj.ap()
                  b_bcast = bass.AP(tensor=b_ap.tensor, offset=b_ap.offset,
                                    ap=[[0, 128]] + list(b_ap.ap))
                  nc.sync.dma_start(out=bias_bc[:], in_=b_bcast)
                  for p in range(NPL):
                      nc.scalar.dma_start(
                          out=wp[p][:],
                          in_=w_proj[p * 128:(p + 1) * 128, :])

                  kt0, qt0, vs0, direct0, fill0 = make_qkv(
                      0, (wk0t, wq0t, wv0t))
                  for ch in direct0:
                      ch()
                  fillers.extend(fill0)
                  cur = (kt0, qt0, vs0)
                  for p in range(NPL):
                      if p + 1 < NPL:
                          k1, q1, v1, direct1, fill1 = make_qkv(
                              p + 1, emit_w(p + 1))
                          fillers.extend(direct1)
                          fillers.extend(fill1)
                          nxt = (k1, q1, v1)
                      else:
                          nxt = None
                      last = p == NPL - 1
                      emit_attn_head(p, 0, *cur, final=False)
                      emit_attn_head(p, 1, *cur, final=last)
                      cur = nxt
                  ps_att.close()
                  with tc.tile_pool(name="ps_tail", bufs=6,
                                    space="PSUM") as ps_tail:
                      ps_proj[0] = ps_tail
                      while fillers:
                          drain_filler()

    nc.compile()
    return nc


def _make_runner(nc):
    """Cached multi-core PJRT runner (mirrors run_bass_via_pjrt, but the
    jitted callable is built once and reused across kernel() calls)."""
    import jax
    import numpy as np
    from jax.experimental.shard_map import shard_map
    from jax.sharding import Mesh, PartitionSpec
    import concourse.mybir as mybir
    from concourse.bass2jax import (_bass_exec_p, install_neuronx_cc_hook,
                                    partition_id_tensor)

    install_neuronx_cc_hook()

    partition_name = (nc.partition_id_tensor.name
                      if nc.partition_id_tensor else None)
    in_names, out_names, out_avals, zero_outs = [], [], [], []
    for alloc in nc.m.functions[0].allocations:
        if not isinstance(alloc, mybir.MemoryLocationSet):
            continue
        name = alloc.memorylocations[0].name
        if alloc.kind == "ExternalInput":
            if name != partition_name:
                in_names.append(name)
        elif alloc.kind == "ExternalOutput":
            shape = tuple(alloc.tensor_shape)
            dtype = mybir.dt.np(alloc.dtype)
            out_names.append(name)
            out_avals.append(jax.core.ShapedArray(shape, dtype))
            zero_outs.append(np.zeros(shape, dtype))
    n_params = len(in_names)
    n_outs = len(out_avals)
    all_in_names = list(in_names) + list(out_names)
    if partition_name is not None:
        all_in_names.append(partition_name)

    def _body(*args):
        operands = list(args)
        if partition_name is not None:
            operands.append(partition_id_tensor())
        outs = _bass_exec_p.bind(
            *operands,
            out_avals=tuple(out_avals),
            in_names=tuple(all_in_names),
            out_names=tuple(out_names),
            lowering_input_output_aliases=(),
            sim_require_finite=True,
            sim_require_nnan=True,
            nc=nc,
        )
        return tuple(outs)

    devices = jax.devices()[:NCORES]
    mesh = Mesh(np.asarray(devices), ("core",))
    in_specs = (PartitionSpec("core"),) * (n_params + n_outs)
    out_specs = (PartitionSpec("core"),) * n_outs
    sharded = jax.jit(
        shard_map(_body, mesh=mesh, in_specs=in_specs, out_specs=out_specs,
                  check_rep=False),
        donate_argnums=tuple(range(n_params, n_params + n_outs)),
        keep_unused=True,
    )

    def run(in_maps):
        per_core = [[np.asarray(m[nm]) for nm in in_names] for m in in_maps]
        concat_in = [
            np.concatenate([per_core[c][i] for c in range(NCORES)], axis=0)
            for i in range(n_params)
        ]
        concat_zeros = [
            np.zeros((NCORES * z.shape[0], *z.shape[1:]), z.dtype)
            for z in zero_outs
        ]
        out_arrs = sharded(*concat_in, *concat_zeros)
        return [
            {nm: np.asarray(out_arrs[i]).reshape(NCORES, *out_avals[i].shape)[c]
             for i, nm in enumerate(out_names)}
            for c in range(NCORES)
        ]

    return run


def _get_runner():
    global _RUNNER
    if _RUNNER is None:
        nc = _build_program()
        _RUNNER = _make_runner(nc)
    return _RUNNER


def _make_in_maps(x, w_qkv, w_proj, b_proj):
    import ml_dtypes
    bf16 = ml_dtypes.bfloat16
    x = np.asarray(x, dtype=np.float32)
    w_qkv = np.asarray(w_qkv, dtype=np.float32)
    w_proj = np.asarray(w_proj, dtype=np.float32)
    b_proj = np.ascontiguousarray(np.asarray(b_proj, dtype=np.float32))
    zeros_b = np.zeros_like(b_proj)
    in_maps = []
    xt_cache = {}
    for c in range(NCORES):
        b, hf = divmod(c, 2)
        if b not in xt_cache:
            xt_cache[b] = np.ascontiguousarray(x[b].T.astype(bf16))
        c0 = hf * WCOLS
        wqkv_c = np.ascontiguousarray(np.concatenate(
            [w_qkv[:, c0:c0 + WCOLS],
             w_qkv[:, D + c0:D + c0 + WCOLS],
             w_qkv[:, 2 * D + c0:2 * D + c0 + WCOLS]],
            axis=1).astype(bf16))
        wproj_c = np.ascontiguousarray(
            w_proj[c0:c0 + WCOLS, :].astype(bf16))
        in_maps.append({"xt": xt_cache[b], "wqkv": wqkv_c,
                        "wproj": wproj_c,
                        "bias": b_proj if hf == 0 else zeros_b})
    return in_maps


def _assemble(results):
    """results: per-core dicts with 'y' [N, D]. Sums head-half partials."""
    out = np.empty((B, N, D), dtype=np.float32)
    for b in range(B):
        out[b] = results[2 * b]["y"] + results[2 * b + 1]["y"]
    return out


def kernel(x, w_qkv, w_proj, b_proj):
    run = _get_runner()
    results = run(_make_in_maps(x, w_qkv, w_proj, b_proj))
    return _assemble(results)


# revision 51
# speedup vs baseline: 2.3173x; 2.3173x over previous
"""Multi-head attention (b=4, n=2048, d=768, h=12) on 8 trn2 NeuronCores.

Sharding: (batch x head-half) -> 8 shards. Each core gets one batch's x
and SIX of the twelve heads (3 head pairs), computes Q/K/V for the full
sequence for just those heads, runs attention + its slice of the output
projection for the full 2048 queries, and returns a partial [2048, 768]
output (bias folded into the even core of each pair). Host sums the two
partials per batch. No collectives, no duplicated K/V compute.

Device algorithm (per core); storage bf16 for all matmul operands (PSUM
accumulation and softmax stay fp32; end-to-end rel err ~2e-3):
  1. x^T supplied pre-transposed (and pre-cast to bf16) by the host.
  2. Per local head pair: K^T/Q^T/V^T = (x @ W)^T via lhsT=W chunks
     ([128, 2048] each); V^T is PE-transposed back to natural [n, d]
     layout (+ ones column per head for the softmax denominators).
  3. Per head, per query-half qh: S^T[k,q] = K^T_slice.T @ Q^T slices,
     P^T = exp(S^T * scale) on ACT (bf16 out), O^T += V_aug.T @ P^T into
     two 1-bank PSUM halves [65,512]. Row 64 of O^T = denominators.
  4. Projection per (head, qh, qblock): denominators land on partitions
     via a K=1 matmul of the O^T denom row, then
     acc[qb] += r_q * (O_un,h @ W_h) via fused scalar_tensor_tensor.
     The first head accumulates onto the broadcast bias, so acc needs no
     seeding and the epilogue is just 16 output DMAs that stream out as
     the last head's projection completes.

Scheduling: the next pair's QKV and the previous (head, qh)'s
projection are chopped into <=1.3us "filler" chunks drained one per
kt-iteration, and O(kt-1) is emitted after S(kt)/exp(kt) (software
pipeline), so the PE works through the exp latency instead of stalling
on it; ACT (exp, the per-head limiter) never starves. TimelineSim:
298us/core vs 353us for the fp32r batch x seq-half baseline.
"""

from collections import deque

import numpy as np

B, N, D, H, HD = 4, 2048, 768, 12, 64
SCALE = HD ** -0.5
NCORES = 8
NT = N // 128           # 16 key tiles
DT = D // 128           # 6 d-chunks
QT = N // 128           # 16 query blocks (full seq per core)
NPL = 3                 # local head pairs per core (6 heads)
WCOLS = NPL * 128       # 384 packed weight columns per section

_RUNNER = None


def _build_program(reps=1):
    import concourse.bass as bass
    import concourse.tile as tile
    import concourse.mybir as mybir
    from concourse import bacc
    from concourse.masks import make_identity
    from contextlib import ExitStack

    f32 = mybir.dt.float32
    bf16 = mybir.dt.bfloat16
    AF = mybir.ActivationFunctionType
    ALU = mybir.AluOpType

    nc = bacc.Bacc("TRN2", target_bir_lowering=False, debug=False,
                   num_devices=NCORES)

    # host passes x^T bf16; per-core packed qkv weights [D, 3*384]
    # (q cols | k cols | v cols for this core's 3 pairs); w_proj rows for
    # this core's heads [384, D]; bias (real on even cores, zeros on odd).
    xt_in = nc.dram_tensor("xt", [D, N], bf16, kind="ExternalInput")
    w_qkv = nc.dram_tensor("wqkv", [D, 3 * WCOLS], bf16, kind="ExternalInput")
    w_proj = nc.dram_tensor("wproj", [WCOLS, D], bf16, kind="ExternalInput")
    b_proj = nc.dram_tensor("bias", [D], f32, kind="ExternalInput")
    y = nc.dram_tensor("y", [N, D], f32, kind="ExternalOutput")

    with tile.TileContext(nc) as tc:
      for _rep in range(reps):
        with ExitStack() as ctx:
          singles = ctx.enter_context(tc.tile_pool(name="singles", bufs=1))
          onat_pool = ctx.enter_context(tc.tile_pool(name="onat", bufs=1))
          small_pool = ctx.enter_context(tc.tile_pool(name="small", bufs=4))
          # psum pool for transposes / QKV accumulation / proj (2 banks)
          ps_misc = ctx.enter_context(
              tc.tile_pool(name="ps_misc", bufs=2, space="PSUM"))

          ident = singles.tile([128, 128], bf16)
          make_identity(nc, ident)

          bias_bc = singles.tile([128, D], f32)

          ones128 = singles.tile([128, 2], bf16)
          nc.vector.memset(ones128[:], 1.0)

          # warm the ACT exp table during the startup DMA window
          warm = small_pool.tile([128, 1], f32, tag="warm", name="warm")
          nc.vector.memset(warm[:], 0.0)
          nc.scalar.activation(warm[:], warm[:], AF.Exp, bias=0.0, scale=1.0)

          # w_proj pair-row tiles [128, D]; DMAs emitted after the x^T
          # loads (see startup section). The first head's projection
          # accumulates onto bias_bc directly, so acc needs no seeding.
          wp = [singles.tile([128, D], bf16, tag=f"wp{p}", name=f"wp{p}")
                for p in range(NPL)]
          acc = [onat_pool.tile([128, D], f32, tag=f"acc{qb}", name=f"acc{qb}")
                 for qb in range(QT)]

          with tc.tile_pool(name="xt", bufs=1) as xt_pool:
              xT = [xt_pool.tile([128, N], bf16, tag=f"xt{dt}", name=f"xt{dt}")
                    for dt in range(DT)]

              with tc.tile_pool(name="wkq", bufs=16) as wkq_pool, \
                   tc.tile_pool(name="kq", bufs=2) as kq_pool, \
                   tc.tile_pool(name="vsb", bufs=2) as vsb_pool, \
                   tc.tile_pool(name="pt", bufs=3) as pt_pool, \
                   tc.tile_pool(name="otsb", bufs=3) as ot_pool:
                  ps_att = ExitStack()
                  ps_st = ps_att.enter_context(
                      tc.tile_pool(name="ps_st", bufs=2, space="PSUM"))
                  ps_ot = ps_att.enter_context(
                      tc.tile_pool(name="ps_ot", bufs=1, space="PSUM"))
                  # proj chunks allocate their PSUM from here; swapped to
                  # a deeper tail pool once the attention PSUM is freed
                  ps_proj = [ps_misc]

                  # ---- weight DMAs for one pair: one DMA per matrix,
                  # landing as [128, DT, 128] (partition = row-within-
                  # d-chunk) so lhsT slices are t[:, dt, :] ----
                  wq_ap = w_qkv.ap()

                  def w_src(col0, dt0, ndt):
                      return bass.AP(
                          tensor=wq_ap.tensor,
                          offset=wq_ap.offset + col0 + dt0 * 128 * 3 * WCOLS,
                          ap=[[3 * WCOLS, 128],
                              [128 * 3 * WCOLS, ndt],
                              [1, 128]])

                  def emit_w(p):
                      out = []
                      for (tag, col0), eng in zip(
                              (("wk", WCOLS + p * 128),
                               ("wq", p * 128),
                               ("wv", 2 * WCOLS + p * 128)),
                              (nc.sync, nc.gpsimd, nc.sync)):
                          t = wkq_pool.tile([128, DT, 128], bf16, tag=tag,
                                            name=tag)
                          eng.dma_start(out=t[:], in_=w_src(col0, 0, DT))
                          out.append(lambda dt, t=t: t[:, dt, :])
                      wk, wq, wv = out
                      return wk, wq, wv

                  # ---- one 512-col slice of a [128, 2048] (x @ W)^T ----
                  def emit_xw_slice(wf, dst, ns):
                      pss = ps_misc.tile([128, 512], f32, tag="misc",
                                         name="pss")
                      for dt in range(DT):
                          nc.tensor.matmul(
                              pss[:], lhsT=wf(dt),
                              rhs=xT[dt][:, ns * 512:(ns + 1) * 512],
                              start=(dt == 0), stop=(dt == DT - 1))
                      nc.vector.tensor_copy(
                          out=dst[:, ns * 512:(ns + 1) * 512], in_=pss[:])

                  # ---- V^T tile-pair transpose into natural layout ----
                  def emit_vt_chunk(VTt, Vsb, kts):
                      for kt in kts:
                          ps = ps_misc.tile([128, 128], bf16, tag="misc")
                          nc.tensor.transpose(
                              ps[:], VTt[:, kt * 128:(kt + 1) * 128],
                              ident[:])
                          dst = Vsb[:, kt, :].rearrange(
                              "n (h c) -> n h c", h=2)
                          nc.vector.tensor_copy(
                              out=dst[:, :, 0:64],
                              in_=ps[:].rearrange("n (h c) -> n h c", h=2))

                  # ---- build the QKV chunk thunks for pair p ----
                  # returns (KT, QTt, Vsb, direct-chunks, filler-chunks);
                  # direct chunks are the minimum needed before the
                  # pair's first attention kt-iterations can run, the
                  # rest interleave as fillers
                  def make_qkv(p, w3):
                      wk, wq, wv = w3
                      KT = kq_pool.tile([128, N], bf16, tag="kt")
                      QTt = kq_pool.tile([128, N], bf16, tag="qt")
                      VTt = kq_pool.tile([128, N], bf16, tag="vt")
                      Vsb = vsb_pool.tile([128, NT, 130], bf16, tag="v")

                      def K(ns):
                          return lambda: emit_xw_slice(wk, KT, ns)

                      def Q(ns):
                          return lambda: emit_xw_slice(wq, QTt, ns)

                      def V(ns):
                          return lambda: emit_xw_slice(wv, VTt, ns)

                      def T(k0):
                          return lambda: emit_vt_chunk(
                              VTt, Vsb, range(k0, k0 + 4))

                      def vsb_ones():
                          nc.vector.memset(Vsb[:, :, 64:65], 1.0)
                          nc.vector.memset(Vsb[:, :, 129:130], 1.0)

                      direct = [K(0), Q(0), Q(1), vsb_ones, V(0)]
                      fill = [T(0), K(1), V(1), T(4), K(2), V(2), T(8),
                              K(3), V(3), T(12), Q(2), Q(3)]
                      return KT, QTt, Vsb, direct, fill

                  fillers = deque()

                  def drain_filler():
                      if fillers:
                          fillers.popleft()()

                  # ---- projection of one (head, qh): 8 qblocks, chopped
                  # into 2-qblock filler chunks. The first head reads
                  # bias_bc as the accumulate input (acc is uninitialized
                  # until then); later heads accumulate onto acc. ----
                  def push_proj(p, hh, otsb, qh, final):
                      first = p == 0 and hh == 0
                      if hh == 0:
                          orows = otsb[0:64, :]
                          drow = otsb[64:65, :]
                          done = ones128[64:65, :]
                      else:
                          orows = otsb[64:128, :]
                          drow = otsb[0:1, :]
                          done = ones128[0:1, :]
                      wpt = wp[p]

                      def emit_proj_qb(qb):
                          qsl = slice(qb * 128, (qb + 1) * 128)
                          psp = ps_proj[0]
                          sden = psp.tile([128, 2], f32, tag="misc",
                                          name="sden")
                          nc.tensor.matmul(sden[:], lhsT=drow[:, qsl],
                                           rhs=done, start=True, stop=True)
                          rcp = small_pool.tile([128, 1], f32, tag="rcp")
                          nc.vector.reciprocal(rcp[:], sden[:, 0:1])
                          # PSUM is only reachable from DVE (not GPSIMD),
                          # so the normalize-accumulate stays on DVE
                          veng = nc.vector
                          for i in range(2):
                              pp = psp.tile([128, 384], f32,
                                            tag="misc", name="pp")
                              nc.tensor.matmul(
                                  pp[:], lhsT=orows[:, qsl],
                                  rhs=wpt[hh * 64:(hh + 1) * 64,
                                          i * 384:(i + 1) * 384],
                                  start=True, stop=True)
                              src = (bias_bc if first else acc[qb])
                              veng.scalar_tensor_tensor(
                                  out=acc[qb][:, i * 384:(i + 1) * 384],
                                  in0=pp[:], scalar=rcp[:],
                                  in1=src[:, i * 384:(i + 1) * 384],
                                  op0=ALU.mult, op1=ALU.add)
                          if final:
                              eng = (nc.sync, nc.scalar, nc.gpsimd)[qb % 3]
                              eng.dma_start(out=y[qb * 128:(qb + 1) * 128, :],
                                            in_=acc[qb][:])

                      for qb0 in range(qh * 8, qh * 8 + 8):
                          fillers.append(
                              lambda qb0=qb0: emit_proj_qb(qb0))

                  # ---- attention for one head; interleaves fillers.
                  # The software pipeline carries across qh/head/pair
                  # boundaries: O(kt) and the previous qh's copy-out +
                  # proj-push are emitted one iteration late, inside the
                  # NEXT qh's kt loop, so the PE never resets at a
                  # boundary ----
                  carry = {"o": None, "fin": None}

                  def emit_attn_head(p, hh, KT, QTt, Vsb, final):
                      base = hh * 64
                      otsb = ot_pool.tile([128, N], bf16, tag="otsb")
                      for qh in range(2):
                          # O^T accumulates into two 1-bank halves so the
                          # next qh's accumulation only waits on the
                          # matching half's copy-out
                          ot0 = ps_ot.tile([65, 512], f32, tag="ot0",
                                           name="ot0")
                          ot1 = ps_ot.tile([65, 512], f32, tag="ot1",
                                           name="ot1")
                          ot = [ot0, ot1]
                          for kt in range(NT):
                              st = ps_st.tile([128, N // 2], f32, tag="st")
                              lhsT = KT[base:base + 64,
                                        kt * 128:(kt + 1) * 128]
                              for i in range(2):
                                  q0 = qh * 1024 + i * 512
                                  nc.tensor.matmul(
                                      st[:, i * 512:(i + 1) * 512],
                                      lhsT=lhsT,
                                      rhs=QTt[base:base + 64, q0:q0 + 512],
                                      start=True, stop=True)
                              pt = pt_pool.tile([128, N // 2], bf16,
                                                tag="pt")
                              nc.scalar.activation(pt[:], st[:], AF.Exp,
                                                   bias=0.0,
                                                   scale=float(SCALE))
                              if final and qh == 1:
                                  if kt >= 8:
                                      drain_filler()
                              else:
                                  drain_filler()
                              if carry["o"] is not None:
                                  carry["o"]()
                                  carry["o"] = None
                              if carry["fin"] is not None:
                                  carry["fin"]()
                                  carry["fin"] = None
                              vh = Vsb[:, kt, hh * 65: hh * 65 + 65]

                              def o_emit(pt=pt, vh=vh, kt=kt, ot=ot):
                                  for i in range(2):
                                      nc.tensor.matmul(
                                          ot[i][:],
                                          lhsT=vh,
                                          rhs=pt[:, i * 512:(i + 1) * 512],
                                          start=(kt == 0),
                                          stop=(kt == NT - 1))
                              carry["o"] = o_emit

                          def fin(qh=qh, ot=ot, otsb=otsb, hh=hh, p=p,
                                  final=final):
                              # pack this qh's O^T into otsb so lhsT/rhs
                              # base partitions line up for the proj
                              # matmuls (final head, last qh: one half on
                              # ACT, which is idle in the tail)
                              def ocopy(out, in_, i=0):
                                  if final and qh == 1 and i == 0:
                                      nc.scalar.activation(
                                          out, in_, AF.Copy,
                                          bias=0.0, scale=1.0)
                                  else:
                                      nc.vector.tensor_copy(out=out,
                                                            in_=in_)
                              for i in range(2):
                                  osl = slice(qh * 1024 + i * 512,
                                              qh * 1024 + (i + 1) * 512)
                                  if hh == 0:
                                      ocopy(otsb[0:65, osl], ot[i][:], i)
                                  else:
                                      ocopy(otsb[64:128, osl],
                                            ot[i][0:64, :], i)
                                      ocopy(otsb[0:1, osl],
                                            ot[i][64:65, :], i)
                              push_proj(p, hh, otsb, qh, final)
                          carry["fin"] = fin

                  # ---- startup DMAs. sync + scalar share the hardware
                  # DGE (~0.63us per DMA, serial); gpsimd runs software
                  # DGE on the Pool engine (~1us per DMA) but is a third
                  # parallel path. Order: pair-0 wk, then x^T column
                  # halves (first-needed first), then wq/wv, bias, wp.
                  def w_dma(tag, col0, eng):
                      t = wkq_pool.tile([128, DT, 128], bf16, tag=tag,
                                        name=tag)
                      eng.dma_start(out=t[:], in_=w_src(col0, 0, DT))
                      return lambda dt, t=t: t[:, dt, :]

                  # pair-0 wk split: a tiny dt0-only tile lands first so
                  # the very first matmul starts earlier
                  wk0a = wkq_pool.tile([128, 1, 128], bf16, tag="wk0a",
                                       name="wk0a")
                  nc.sync.dma_start(out=wk0a[:], in_=w_src(WCOLS, 0, 1))
                  wk0b = wkq_pool.tile([128, DT - 1, 128], bf16, tag="wk",
                                       name="wk0b")
                  nc.sync.dma_start(out=wk0b[:], in_=w_src(WCOLS, 1, DT - 1))

                  def wk0t(dt):
                      return wk0a[:, 0, :] if dt == 0 else wk0b[:, dt - 1, :]

                  for ns in range(4):
                      for dt in range(DT):
                          eng = (nc.gpsimd, nc.sync, nc.scalar)[dt % 3]
                          eng.dma_start(
                              out=xT[dt][:, ns * 512:(ns + 1) * 512],
                              in_=xt_in[dt * 128:(dt + 1) * 128,
                                        ns * 512:(ns + 1) * 512])
                      if ns == 0:
                          wq0t = w_dma("wq", 0, nc.gpsimd)
                          wv0t = w_dma("wv", 2 * WCOLS, nc.gpsimd)
                  b_ap = b_proj.ap()
                  b_bcast = bass.AP(tensor=b_ap.tensor, offset=b_ap.offset,
                                    ap=[[0, 128]] + list(b_ap.ap))
                  nc.sync.dma_start(out=bias_bc[:], in_=b_bcast)
                  for p in range(NPL):
                      nc.scalar.dma_start(
                          out=wp[p][:],
                          in_=w_proj[p * 128:(p + 1) * 128, :])

                  kt0, qt0, vs0, direct0, fill0 = make_qkv(
                      0, (wk0t, wq0t, wv0t))
                  for ch in direct0:
                      ch()
                  fillers.extend(fill0)
                  cur = (kt0, qt0, vs0)
                  for p in range(NPL):
                      if p + 1 < NPL:
                          k1, q1, v1, direct1, fill1 = make_qkv(
                              p + 1, emit_w(p + 1))
                          fillers.extend(direct1)
                          fillers.extend(fill1)
                          nxt = (k1, q1, v1)
                      else:
                          nxt = None
                      last = p == NPL - 1
                      emit_attn_head(p, 0, *cur, final=False)
                      emit_attn_head(p, 1, *cur, final=last)
                      cur = nxt
                  carry["o"]()
                  carry["fin"]()
                  ps_att.close()
                  with tc.tile_pool(name="ps_tail", bufs=6,
                                    space="PSUM") as ps_tail:
                      ps_proj[0] = ps_tail
                      while fillers:
                          drain_filler()

    nc.compile()
    return nc


def _make_runner(nc):
    """Cached multi-core PJRT runner (mirrors run_bass_via_pjrt, but the
    jitted callable is built once and reused across kernel() calls)."""
    import jax
    import numpy as np
    from jax.experimental.shard_map import shard_map
    from jax.sharding import Mesh, PartitionSpec
    import concourse.mybir as mybir
    from concourse.bass2jax import (_bass_exec_p, install_neuronx_cc_hook,
                                    partition_id_tensor)

    install_neuronx_cc_hook()

    partition_name = (nc.partition_id_tensor.name
                      if nc.partition_id_tensor else None)
    in_names, out_names, out_avals, zero_outs = [], [], [], []
    for alloc in nc.m.functions[0].allocations:
        if not isinstance(alloc, mybir.MemoryLocationSet):
            continue
        name = alloc.memorylocations[0].name
        if alloc.kind == "ExternalInput":
            if name != partition_name:
                in_names.append(name)
        elif alloc.kind == "ExternalOutput":
            shape = tuple(alloc.tensor_shape)
            dtype = mybir.dt.np(alloc.dtype)
            out_names.append(name)
            out_avals.append(jax.core.ShapedArray(shape, dtype))
            zero_outs.append(np.zeros(shape, dtype))
    n_params = len(in_names)
    n_outs = len(out_avals)
    all_in_names = list(in_names) + list(out_names)
    if partition_name is not None:
        all_in_names.append(partition_name)

    def _body(*args):
        operands = list(args)
        if partition_name is not None:
            operands.append(partition_id_tensor())
        outs = _bass_exec_p.bind(
            *operands,
            out_avals=tuple(out_avals),
            in_names=tuple(all_in_names),
            out_names=tuple(out_names),
            lowering_input_output_aliases=(),
            sim_require_finite=True,
            sim_require_nnan=True,
            nc=nc,
        )
        return tuple(outs)

    devices = jax.devices()[:NCORES]
    mesh = Mesh(np.asarray(devices), ("core",))
    in_specs = (PartitionSpec("core"),) * (n_params + n_outs)
    out_specs = (PartitionSpec("core"),) * n_outs
    sharded = jax.jit(
        shard_map(_body, mesh=mesh, in_specs=in_specs, out_specs=out_specs,
                  check_rep=False),
        donate_argnums=tuple(range(n_params, n_params + n_outs)),
        keep_unused=True,
    )

    def run(in_maps):
        per_core = [[np.asarray(m[nm]) for nm in in_names] for m in in_maps]
        concat_in = [
            np.concatenate([per_core[c][i] for c in range(NCORES)], axis=0)
            for i in range(n_params)
        ]
        concat_zeros = [
            np.zeros((NCORES * z.shape[0], *z.shape[1:]), z.dtype)
            for z in zero_outs
        ]
        out_arrs = sharded(*concat_in, *concat_zeros)
        return [
            {nm: np.asarray(out_arrs[i]).reshape(NCORES, *out_avals[i].shape)[c]
             for i, nm in enumerate(out_names)}
            for c in range(NCORES)
        ]

    return run


def _get_runner():
    global _RUNNER
    if _RUNNER is None:
        nc = _build_program()
        _RUNNER = _make_runner(nc)
    return _RUNNER


def _make_in_maps(x, w_qkv, w_proj, b_proj):
    import ml_dtypes
    bf16 = ml_dtypes.bfloat16
    x = np.asarray(x, dtype=np.float32)
    w_qkv = np.asarray(w_qkv, dtype=np.float32)
    w_proj = np.asarray(w_proj, dtype=np.float32)
    b_proj = np.ascontiguousarray(np.asarray(b_proj, dtype=np.float32))
    zeros_b = np.zeros_like(b_proj)
    in_maps = []
    xt_cache = {}
    for c in range(NCORES):
        b, hf = divmod(c, 2)
        if b not in xt_cache:
            xt_cache[b] = np.ascontiguousarray(x[b].T.astype(bf16))
        c0 = hf * WCOLS
        wqkv_c = np.ascontiguousarray(np.concatenate(
            [w_qkv[:, c0:c0 + WCOLS],
             w_qkv[:, D + c0:D + c0 + WCOLS],
             w_qkv[:, 2 * D + c0:2 * D + c0 + WCOLS]],
            axis=1).astype(bf16))
        wproj_c = np.ascontiguousarray(
            w_proj[c0:c0 + WCOLS, :].astype(bf16))
        in_maps.append({"xt": xt_cache[b], "wqkv": wqkv_c,
                        "wproj": wproj_c,
                        "bias": b_proj if hf == 0 else zeros_b})
    return in_maps


def _assemble(results):
    """results: per-core dicts with 'y' [N, D]. Sums head-half partials."""
    out = np.empty((B, N, D), dtype=np.float32)
    for b in range(B):
        out[b] = results[2 * b]["y"] + results[2 * b + 1]["y"]
    return out


def kernel(x, w_qkv, w_proj, b_proj):
    run = _get_runner()
    results = run(_make_in_maps(x, w_qkv, w_proj, b_proj))
    return _assemble(results)
